# revision 1
# baseline (speedup 1.0000x reference)
"""A-trous cross-bilateral filter (5x5 B3-spline stencil, numIter dilated passes)
on 8 TRN2 NeuronCores.

Sharding: host-side. The full [B,C,H,W] problem is cut into 8 overlapping
shards (batch x H-half, each with a `pad0 = 2*(2^n - 1)` halo taken from the
edge-padded full image), so each core computes its output slice with zero
cross-core communication.

Per-core kernel: for each iteration (dilation d = 1,2,4,8), stream row blocks
through SBUF. All 25 stencil shifts are plain AP offsets inside a
[rows, ch, W] tile. Per unique tap (12 after point symmetry):
  sq   = (g - g_shift)^2           (DVE, all 4 guidance channels packed)
  D    = sum_p sq                  (TensorE: 4 identity-matmul accumulates
                                    into PSUM, or DVE adds)
  w    = exp(-D + ln K2)           (ScalarE, bias folds the kernel coeff)
then the tap and its mirror accumulate w*x (3 channels) and w (denominator)
into PSUM via identity-matmul accumulation. The center tap is a scaled
identity matmul. Finally out = num * recip(den) and the block is stored to a
DRAM intermediate (bf16) for the next iteration.
"""

import math
from contextlib import ExitStack
from dataclasses import dataclass

import numpy as np

_K1 = np.array([1.0 / 16, 1.0 / 4, 3.0 / 8, 1.0 / 4, 1.0 / 16], dtype=np.float64)
_K2 = np.outer(_K1, _K1)  # [5,5]


def _pads(n):
    # pad remaining before iteration k (k=0..n); consumed 2*d per side per pass
    return [2 * ((1 << n) - (1 << k)) for k in range(n + 1)]


@dataclass
class Cfg:
    Hout: int = 512          # per-core output rows
    Wout: int = 1024         # output cols (not sharded)
    C: int = 3
    P: int = 4
    numIter: int = 4
    dt_elem: str = "bfloat16"    # elementwise/storage dtype: "bfloat16"|"float32"
    sqdiff: str = "tt"           # "tt" (sub+mult) | "custom" (fused DVE op) | "act" (sub + ACT square)
    d_reduce: str = "pe"         # "pe" | "dve"
    recip: str = "fast"          # "fast" | "exact"
    bcast_prod: bool = True      # w*x products via partition-bcast-free 3ch packed op
    reps: int = 1                # repeat whole pipeline (timing)
    # Reference semantics re-apply edge padding to the intermediate x each
    # iteration. Shards are arranged so the global row-edge is the TOP of
    # every shard (bottom halves are flipped on the host); between iterations
    # we re-replicate the top row band and both col bands of the DRAM
    # intermediate. replicate_bottom is for single-shard (full-image) tests.
    replicate_bottom: bool = False
    tap_filter: tuple = ()   # debug: restrict to these (dy,dx) unique taps
    report: bool = False     # print analytic per-engine busy estimate
    init: str = "pe"         # accumulator init: "pe" (scaled-identity matmul) | "dve"
    sqdiff_perf: bool = False  # enable 2x perf-mode slots for the custom sqdiff
    sq_act: int = 6          # first N unique taps use sub+ACT-square instead of sqdiff mode
    greedy_chunk: bool = False  # W chunks of max size instead of even split
    gp_prod: int = 0         # this many mirror-product ops per chunk go to GPSIMD
    # cache exp(-D) fields: iter k's even-offset taps are iter k+1's inner
    # 3x3, and g is constant across iterations, so those 4 unique weight
    # fields can be stored to DRAM and re-loaded instead of recomputed.
    w_cache: bool = False
    io_bufs: int = 2
    work_bufs: int = 3
    psum_d_bufs: int = 3


class _Est:
    """Analytic per-engine busy estimate (ns), from the documented TRN2
    cost formulas. DVE: (58+FD/acc)/0.96; ACT: (224+FD)/1.2;
    PE: max(60, 6+FD)/2.4; DMA: bytes at ~185 GB/s/core aggregate."""

    def __init__(self):
        self.ns = {"DVE": 0.0, "ACT": 0.0, "PE": 0.0, "DMA": 0.0}
        self.cnt = {"DVE": 0, "ACT": 0, "PE": 0, "DMA": 0}

    def dve(self, fd, acc=1, n=1):
        self.ns["DVE"] += n * (58 + fd / acc) / 0.96
        self.cnt["DVE"] += n

    def act(self, fd, n=1):
        self.ns["ACT"] += n * (224 + fd) / 1.2
        self.cnt["ACT"] += n

    def pe(self, fd, n=1):
        self.ns["PE"] += n * max(60, 6 + fd) / 2.4
        self.cnt["PE"] += n

    def dma(self, nbytes, n=1):
        self.ns["DMA"] += nbytes / 185.0
        self.cnt["DMA"] += n

    def show(self, label):
        parts = ", ".join(
            f"{k}={self.ns[k]/1e3:.0f}us/{self.cnt[k]}" for k in self.ns
        )
        print(f"[est {label}] {parts}", flush=True)


def get_sqdiff_op(perf: bool = False):
    """Register a fused (a-b)^2 custom DVE op; sha pinned at first use."""
    from concourse import dve_ops
    from concourse.dve_spec import Spec, Src0, Src1, sq, lower, _has_src1
    from concourse.dve_uop import DveOpSpec

    name = "SQDIFF2X_ANT" if perf else "SQDIFF_ANT"
    for op in dve_ops.OPS:
        if op.name == name:
            return op
    spec = Spec(
        body=sq(Src0 - Src1),
        reference=lambda in0, in1, c0, c1, c2: (
            in0.astype(np.float32) - in1.astype(np.float32)
        )
        ** 2,
    )
    row = dve_ops._CUSTOM_DVE_ROW_BASE + len(dve_ops.OPS)
    assert row < 0x20
    shas = {}
    for ver in ("v3", "v4"):
        compiled = DveOpSpec(
            name=name, opcode=row, uops=lower(spec, ver=ver), rd1_en=_has_src1(spec)
        )
        shas[ver] = compiled.sha(ver)
    perf_en = {"v3": True, "v4": True} if perf else {}
    op = dve_ops.DveOp(name, spec, subdim=False, uops_sha=shas, perf_en=perf_en)
    dve_ops.OPS.append(op)
    dve_ops.CUSTOM_DVE_SPECS[name] = spec
    dve_ops._SUB_OPCODE_FOR_NAME[name] = row
    return op


def build(cfg: Cfg):
    """Build the per-core bass graph. Inputs 'x' [C,H0,W0], 'g' [P,H0,W0]
    (dt_elem), output 'out' [C,Hout,Wout] (f32)."""
    import concourse.bass as bass
    import concourse.tile as tile
    from concourse import bacc, mybir
    from concourse.masks import make_identity

    C, P, n = cfg.C, cfg.P, cfg.numIter
    assert 1 <= n <= 5
    pads = _pads(n)
    H = [cfg.Hout + 2 * p for p in pads]
    W = [cfg.Wout + 2 * p for p in pads]
    dt = getattr(mybir.dt, cfg.dt_elem)
    f32 = mybir.dt.float32
    AF = mybir.ActivationFunctionType
    OP = mybir.AluOpType

    sq_op = (
        get_sqdiff_op(cfg.sqdiff_perf)
        if cfg.sqdiff == "custom"
        else None
    )

    est = _Est()
    esz = 2 if cfg.dt_elem == "bfloat16" else 4
    acc2 = 2 if cfg.dt_elem == "bfloat16" else 1
    nc = bacc.Bacc("TRN2", target_bir_lowering=False, debug=False, num_devices=8)
    x_ext = nc.declare_dram_parameter("x", [C, H[0], W[0]], dt, isOutput=False)
    g_ext = nc.declare_dram_parameter("g", [P, H[0], W[0]], dt, isOutput=False)
    out_ext = nc.declare_dram_parameter("out", [C, cfg.Hout, cfg.Wout], f32, isOutput=True)

    # max extension of the weight field beyond a W-chunk: 2*d_max each side
    d_max = 1 << (n - 1)
    WE_MAX = 512  # psum bank limit for D / chunk accumulators

    with tile.TileContext(nc) as tc, ExitStack() as ctx:
        consts = ctx.enter_context(tc.tile_pool(name="consts", bufs=1))
        io = ctx.enter_context(tc.tile_pool(name="io", bufs=cfg.io_bufs))
        work = ctx.enter_context(tc.tile_pool(name="work", bufs=cfg.work_bufs))
        psum_acc = ctx.enter_context(tc.tile_pool(name="psum_acc", bufs=1, space="PSUM"))
        psum_d = ctx.enter_context(tc.tile_pool(name="psum_d", bufs=cfg.psum_d_bufs, space="PSUM"))
        dram = ctx.enter_context(tc.tile_pool(name="dram", bufs=1, space="DRAM"))

        # scaled shifted identities: sid(s, v)[k, m] = v iff k == m + s.
        # matmul(out, lhsT=sid(s, v), rhs) computes out[m] += v * rhs[m+s]:
        # the TensorEngine does both the partition (row) shift (compute
        # engines can only address SBUF operands at base partition 0/32/64/96)
        # and the K2 coefficient scaling for free.
        _sid = {}

        def sid(s, v):
            key = (s, round(v * 256))
            if key not in _sid:
                i = len(_sid)
                t = consts.tile([128, 128], dt, tag=f"sid{i}", name=f"sid{i}")
                nc.gpsimd.memset(t, 0.0)
                nc.gpsimd.affine_select(
                    out=t,
                    in_=t,
                    compare_op=mybir.AluOpType.not_equal,
                    fill=float(v),
                    base=-s,
                    pattern=[[-1, 128]],
                    channel_multiplier=1,
                )
                _sid[key] = t
            return _sid[key]

        ident = sid(0, 1.0)
        # pre-create every (shift, scale) combination used by the tap loops
        # so no const-tile setup ops land mid-pipeline.
        sid(0, float(_K2[2, 2]))
        for it0 in range(n):
            d0 = 1 << it0
            for dy in range(0, 3):
                for dx in range(-2, 3):
                    if dy > 0 or dx > 0:
                        v0 = float(_K2[2 + dy, 2 + dx])
                        sid(dy * d0, v0)
                        sid(0, v0)
        ones = consts.tile([128, WE_MAX], dt, tag="ones")
        nc.vector.memset(ones, 1.0)

        inter = [
            dram.tile([C, H[k], W[k]], dt, tag=f"inter{k}", name=f"inter{k}")
            for k in range(1, n)
        ]

        # exp(-D) field cache: iter it's even-offset unique taps are iter
        # it+1's inner-3x3 unique taps (same offsets, g fixed). Arrays are
        # indexed on iter-it's output grid with a (-2d, -2d) origin shift so
        # all stores/loads keep SBUF base partition 0.
        EVENS = ((0, 2), (2, 0), (2, 2), (2, -2))
        REUSE = {(0, 1): (0, 2), (1, 0): (2, 0), (1, 1): (2, 2), (1, -1): (2, -2)}
        ecache = {}
        if cfg.w_cache:
            for k in range(n - 1):
                dk = 1 << k
                for tkey in EVENS:
                    ecache[(k, tkey)] = dram.tile(
                        [2 * dk + H[k + 1], 4 * dk + W[k + 1]],
                        dt,
                        tag=f"ec{k}_{tkey[0]}_{tkey[1]}",
                        name=f"ec{k}_{tkey[0]}_{tkey[1]}".replace("-", "m"),
                    )

        for _rep in range(cfg.reps):
            for it in range(n):
                d = 1 << it
                src = x_ext if it == 0 else inter[it - 1]
                dst = out_ext if it == n - 1 else inter[it]
                Ws = W[it]
                Ho, Wo = H[it + 1], W[it + 1]
                gofs = pads[0] - pads[it]
                dt_out = f32 if it == n - 1 else dt

                Rmax = 128 - 4 * d
                nb = math.ceil(Ho / Rmax)
                Rblk = math.ceil(Ho / nb)
                Wc_max = WE_MAX - 2 * d
                ncw = math.ceil(Wo / Wc_max)
                Wchunk = Wc_max if cfg.greedy_chunk else math.ceil(Wo / ncw)

                taps = [
                    (dy, dx)
                    for dy in range(0, 3)
                    for dx in range(-2, 3)
                    if (dy > 0 or dx > 0)
                ]
                if d == 1:
                    # odd-dx taps first: their shifted bf16 operands are
                    # 2B-misaligned (DVE falls to 1x), so let sq_act route
                    # them to the ScalarE square path.
                    taps.sort(key=lambda t: abs(t[1]) % 2 == 0)
                if cfg.tap_filter:
                    taps = [t for t in taps if t in cfg.tap_filter]

                for b in range(nb):
                    r0 = b * Rblk
                    R = min(Rblk, Ho - r0)
                    # Row-aligned views (tile row p <-> output row p - base):
                    #   x0/g0 base 0, xm1/g1 base -d, xm2/g2 base -2d.
                    # src row of output row q is q + 2d; g row adds gofs.
                    x0 = io.tile([128, C, Ws], dt, tag="x0")
                    xm1 = io.tile([128, C, Ws], dt, tag="xm1")
                    xm2 = io.tile([128, C, Ws], dt, tag="xm2")
                    g0 = io.tile([128, P, Ws], dt, tag="g0")
                    g1 = io.tile([128, P, Ws], dt, tag="g1")
                    g2 = io.tile([128, P, Ws], dt, tag="g2")
                    nc.sync.dma_start(
                        out=x0[: R + 2 * d],
                        in_=src[:, r0 + 2 * d : r0 + 3 * d + R + d, :].rearrange(
                            "c r w -> r c w"
                        ),
                    )
                    nc.sync.dma_start(
                        out=xm1[:R],
                        in_=src[:, r0 + d : r0 + d + R, :].rearrange("c r w -> r c w"),
                    )
                    nc.sync.dma_start(
                        out=xm2[:R],
                        in_=src[:, r0 : r0 + R, :].rearrange("c r w -> r c w"),
                    )
                    gr = gofs + r0
                    nc.sync.dma_start(
                        out=g0[: R + 2 * d],
                        in_=g_ext[
                            :, gr + 2 * d : gr + 2 * d + R + 2 * d, gofs : gofs + Ws
                        ].rearrange("c r w -> r c w"),
                    )
                    nc.sync.dma_start(
                        out=g1[: R + d],
                        in_=g_ext[
                            :, gr + d : gr + d + R + d, gofs : gofs + Ws
                        ].rearrange("c r w -> r c w"),
                    )
                    nc.sync.dma_start(
                        out=g2[: R + 2 * d],
                        in_=g_ext[
                            :, gr : gr + R + 2 * d, gofs : gofs + Ws
                        ].rearrange("c r w -> r c w"),
                    )
                    gdy = {0: g0, 1: g1, 2: g2}
                    xmdy = {0: x0, 1: xm1, 2: xm2}
                    est.dma(((3 * R + 2 * d) * C + (3 * R + 5 * d) * P) * Ws * esz, n=6)

                    for wci in range(ncw):
                        c0 = wci * Wchunk
                        Wc = min(Wchunk, Wo - c0)
                        num = psum_acc.tile([128, C, WE_MAX], f32, tag="num")
                        den = psum_acc.tile([128, WE_MAX], f32, tag="den")

                        # center tap initializes the accumulators:
                        # num = K2c * x, den = K2c
                        if cfg.init == "pe":
                            identC = sid(0, float(_K2[2, 2]))
                            for c in range(C):
                                nc.tensor.matmul(
                                    num[:R, c, :Wc],
                                    identC[:R, :R],
                                    x0[:R, c, 2 * d + c0 : 2 * d + c0 + Wc],
                                    start=True,
                                    stop=False,
                                    skip_group_check=True,
                                )
                            nc.tensor.matmul(
                                den[:R, :Wc],
                                identC[:R, :R],
                                ones[:R, :Wc],
                                start=True,
                                stop=False,
                                skip_group_check=True,
                            )
                            est.pe(Wc, n=4)
                        else:
                            nc.vector.tensor_scalar(
                                num[:R, :, :Wc],
                                x0[:R, :, 2 * d + c0 : 2 * d + c0 + Wc],
                                scalar1=float(_K2[2, 2]),
                                scalar2=None,
                                op0=OP.mult,
                            )
                            nc.vector.memset(den[:R, :Wc], float(_K2[2, 2]))
                            est.dve(C * Wc, acc=1)
                            est.dve(Wc, acc=2)

                        comp_i = 0
                        for ti, (dy, dx) in enumerate(taps):
                            last_tap = ti == len(taps) - 1
                            oy, ox = dy * d, dx * d
                            Re = R + oy
                            ce = c0 - max(ox, 0)
                            We = Wc + abs(ox)
                            k2v = float(_K2[2 + dy, 2 + dx])
                            # E field exp(-D) over rows [-oy, R), cols
                            # [ce, ce+We) (output coords); tile base row -oy.
                            # The K2 coefficient rides on the accumulation
                            # lhsT, so cached fields are coefficient-free.
                            wt = work.tile([128, WE_MAX + 2 * d_max * 2], dt, tag="wt")
                            reuse = (
                                cfg.w_cache and it > 0 and (dy, dx) in REUSE
                            )
                            if reuse:
                                ea = ecache[(it - 1, REUSE[(dy, dx)])]
                                nc.sync.dma_start(
                                    out=wt[:Re, :We],
                                    in_=ea[
                                        r0 - oy + 3 * d : r0 + R + 3 * d,
                                        ce + 3 * d : ce + We + 3 * d,
                                    ],
                                )
                                est.dma(Re * We * esz)
                            else:
                                gc = gdy[dy][:Re, :, 2 * d + ce : 2 * d + ce + We]
                                gs = g0[:Re, :, 2 * d + ce + ox : 2 * d + ce + ox + We]
                                sq_t = work.tile([128, P, WE_MAX + 2 * d_max * 2], dt, tag="sq")
                                sq_ap = sq_t[:Re, :, :We]
                                sq_mode = "act" if comp_i < cfg.sq_act else cfg.sqdiff
                                comp_i += 1
                                if sq_mode == "custom":
                                    nc.vector._custom_dve(sq_op, out=sq_ap, in0=gc, in1=gs)
                                    est.dve(P * We, acc=2 if cfg.sqdiff_perf else 1)
                                else:
                                    nc.vector.tensor_tensor(sq_ap, gc, gs, op=OP.subtract)
                                    est.dve(P * We, acc=acc2)
                                    if sq_mode == "act":
                                        nc.scalar.activation(sq_ap, sq_ap, AF.Square)
                                        est.act(P * We)
                                    else:
                                        nc.vector.tensor_tensor(sq_ap, sq_ap, sq_ap, op=OP.mult)
                                        est.dve(P * We, acc=acc2)

                                if cfg.d_reduce == "pe":
                                    Dp = psum_d.tile([128, WE_MAX], f32, tag="D")
                                    for p in range(P):
                                        nc.tensor.matmul(
                                            Dp[:Re, :We],
                                            ident[:Re, :Re],
                                            sq_t[:Re, p, :We],
                                            start=(p == 0),
                                            stop=(p == P - 1),
                                        )
                                    d_ap = Dp[:Re, :We]
                                    est.pe(We, n=4)
                                else:
                                    ds = work.tile([128, WE_MAX + 2 * d_max * 2], f32, tag="ds")
                                    nc.vector.tensor_tensor(
                                        ds[:Re, :We], sq_t[:Re, 0, :We], sq_t[:Re, 1, :We], op=OP.add
                                    )
                                    nc.vector.tensor_tensor(
                                        sq_t[:Re, 2, :We], sq_t[:Re, 2, :We], sq_t[:Re, 3, :We], op=OP.add
                                    )
                                    nc.vector.tensor_tensor(
                                        ds[:Re, :We], ds[:Re, :We], sq_t[:Re, 2, :We], op=OP.add
                                    )
                                    d_ap = ds[:Re, :We]
                                    est.dve(We, acc=acc2, n=3)

                                nc.scalar.activation(
                                    wt[:Re, :We], d_ap, AF.Exp, scale=-1.0,
                                )
                                est.act(We)
                                if cfg.w_cache and it < n - 1 and (dy, dx) in EVENS:
                                    ea = ecache[(it, (dy, dx))]
                                    nc.sync.dma_start(
                                        out=ea[
                                            r0 - oy + 2 * d : r0 + R + 2 * d,
                                            ce + 2 * d : ce + We + 2 * d,
                                        ],
                                        in_=wt[:Re, :We],
                                    )
                                    est.dma(Re * We * esz)

                            # direct tap (+oy,+ox): num[q] += w[q] * x[q+t].
                            # products in the shifted frame p = q + oy:
                            #   prod_d[p, v] = wt[p, c0+v] * x0[p, c0+v+ox]
                            # then the shifted-identity matmul folds rows
                            # p = m + oy into output row m.
                            w_d = wt[:Re, c0 - ce : c0 - ce + Wc]
                            x_d = x0[:Re, :, 2 * d + c0 + ox : 2 * d + c0 + ox + Wc]
                            # mirror tap (-oy,-ox): num[q] += w[q-t] * x[q-t],
                            # already base-aligned: wt row q <-> w[q - oy].
                            w_m = wt[:R, c0 - ox - ce : c0 - ox - ce + Wc]
                            x_m = xmdy[dy][:R, :, 2 * d + c0 - ox : 2 * d + c0 - ox + Wc]
                            prod = work.tile([128, 2, C, WE_MAX], dt, tag="prod")
                            for j, (wv, xv, rr) in enumerate(
                                ((w_d, x_d, Re), (w_m, x_m, R))
                            ):
                                if j == 1 and ti < cfg.gp_prod:
                                    wb = bass.AP(
                                        tensor=wv.tensor,
                                        offset=wv.offset,
                                        ap=[wv.ap[0], [0, C], wv.ap[1]],
                                    )
                                    nc.gpsimd.tensor_tensor(
                                        prod[:rr, j, :, :Wc], wb, xv, op=OP.mult
                                    )
                                    continue
                                if cfg.bcast_prod:
                                    wb = bass.AP(
                                        tensor=wv.tensor,
                                        offset=wv.offset,
                                        ap=[wv.ap[0], [0, C], wv.ap[1]],
                                    )
                                    nc.vector.tensor_tensor(
                                        prod[:rr, j, :, :Wc], wb, xv, op=OP.mult
                                    )
                                    est.dve(C * Wc, acc=acc2)
                                else:
                                    for c in range(C):
                                        nc.vector.tensor_tensor(
                                            prod[:rr, j, c, :Wc], wv, xv[:, c, :], op=OP.mult
                                        )
                                        est.dve(Wc, acc=acc2)
                            for j, (wv, sh, rr) in enumerate(
                                ((w_d, oy, Re), (w_m, 0, R))
                            ):
                                stop = last_tap and j == 1
                                lhsT = sid(sh, k2v)
                                for c in range(C):
                                    nc.tensor.matmul(
                                        num[:R, c, :Wc],
                                        lhsT[:rr, :R],
                                        prod[:rr, j, c, :Wc],
                                        start=False,
                                        stop=stop,
                                        skip_group_check=True,
                                    )
                                nc.tensor.matmul(
                                    den[:R, :Wc],
                                    lhsT[:rr, :R],
                                    wv,
                                    start=False,
                                    stop=stop,
                                    skip_group_check=True,
                                )
                                est.pe(Wc, n=4)

                        rden = work.tile([128, WE_MAX], f32, tag="rden")
                        if cfg.recip == "fast":
                            nc.vector.reciprocal_approx_fast(rden[:R, :Wc], den[:R, :Wc])
                        else:
                            nc.vector.reciprocal(rden[:R, :Wc], den[:R, :Wc])
                        est.dve(Wc, acc=1)
                        outt = io.tile([128, C, WE_MAX], dt_out, tag="outt")
                        rsl = rden[:R, :Wc]
                        rb = bass.AP(
                            tensor=rsl.tensor,
                            offset=rsl.offset,
                            ap=[list(rsl.ap[0]), [0, C], list(rsl.ap[1])],
                        )
                        nc.vector.tensor_tensor(
                            outt[:R, :, :Wc], num[:R, :, :Wc], rb, op=OP.mult
                        )
                        est.dve(C * Wc, acc=1)
                        # reference semantics: out-of-image cols of the
                        # intermediate are edge replicas. Overwrite the bands
                        # in-tile (bias-broadcast copy) before storing.
                        if it < n - 1 and pads[it + 1] > 0:
                            pn = pads[it + 1]
                            if wci == 0:
                                for c in range(C):
                                    nc.scalar.activation(
                                        outt[:R, c, 0:pn], outt[:R, c, 0:pn],
                                        AF.Identity, scale=0.0,
                                        bias=outt[:R, c, pn : pn + 1],
                                    )
                            if wci == ncw - 1:
                                for c in range(C):
                                    nc.scalar.activation(
                                        outt[:R, c, Wc - pn : Wc], outt[:R, c, Wc - pn : Wc],
                                        AF.Identity, scale=0.0,
                                        bias=outt[:R, c, Wc - pn - 1 : Wc - pn],
                                    )
                        nc.sync.dma_start(
                            out=dst[:, r0 : r0 + R, c0 : c0 + Wc].rearrange(
                                "c r w -> r c w"
                            ),
                            in_=outt[:R, :, :Wc],
                        )
                        est.dma(R * C * Wc * (4 if it == n - 1 else esz))

                # reference semantics: out-of-image rows of the intermediate
                # are edge replicas (row-broadcast DMA; innermost dim stays
                # contiguous so DGE accepts it). Cols were fixed in-tile.
                if it < n - 1:
                    pk = pads[it + 1]
                    if pk > 0:
                        def _bcast(src_ap, axis_idx, count):
                            ap = [list(a) for a in src_ap.ap]
                            ap[axis_idx] = [0, count]
                            return bass.AP(tensor=src_ap.tensor, offset=src_ap.offset, ap=ap)

                        # top rows [0, pk) := row pk
                        nc.sync.dma_start(
                            out=dst[:, 0:pk, :],
                            in_=_bcast(dst[:, pk : pk + 1, :], 1, pk),
                        )
                        if cfg.replicate_bottom:
                            nc.sync.dma_start(
                                out=dst[:, Ho - pk : Ho, :],
                                in_=_bcast(dst[:, Ho - pk - 1 : Ho - pk, :], 1, pk),
                            )

    if cfg.report:
        est.show(f"n={n} dt={cfg.dt_elem} sqdiff={cfg.sqdiff} d_reduce={cfg.d_reduce} reps={cfg.reps}")
    nc.compile()
    return nc


# ---------------------------------------------------------------------------
# host side
# ---------------------------------------------------------------------------


def _shard_and_run(inp: np.ndarray, param: np.ndarray, cfg: Cfg):
    import ml_dtypes
    from concourse.bass_utils import run_bass_kernel_spmd

    B, C, Hfull, Wfull = inp.shape
    n = cfg.numIter
    pad0 = _pads(n)[0]
    nh = 8 // B
    assert nh in (1, 2), "sharding assumes each shard has at most one global row-edge"
    Hsh = Hfull // nh
    assert Hsh == cfg.Hout and Wfull == cfg.Wout

    np_dt = np.float32 if cfg.dt_elem == "float32" else ml_dtypes.bfloat16
    xp = np.pad(inp, ((0, 0), (0, 0), (pad0, pad0), (pad0, pad0)), mode="edge").astype(np_dt)
    gp = np.pad(param, ((0, 0), (0, 0), (pad0, pad0), (pad0, pad0)), mode="edge").astype(np_dt)

    in_maps = []
    for b in range(B):
        for h in range(nh):
            r = h * Hsh
            xs = xp[b, :, r : r + Hsh + 2 * pad0, :]
            gs = gp[b, :, r : r + Hsh + 2 * pad0, :]
            if h == nh - 1 and nh > 1:
                # flip so the global row-edge is at the top of the shard
                xs, gs = xs[:, ::-1, :], gs[:, ::-1, :]
            in_maps.append(
                {"x": np.ascontiguousarray(xs), "g": np.ascontiguousarray(gs)}
            )

    nc = build(cfg)
    res = run_bass_kernel_spmd(nc, in_maps, core_ids=list(range(8)))

    out = np.empty((B, C, Hfull, Wfull), dtype=np.float32)
    for b in range(B):
        for h in range(nh):
            o = res.results[b * nh + h]["out"]
            if h == nh - 1 and nh > 1:
                o = o[:, ::-1, :]
            out[b, :, h * Hsh : (h + 1) * Hsh, :] = o
    return out


def kernel(input: np.ndarray, param: np.ndarray, numIter) -> np.ndarray:
    n = int(numIter)
    inp = np.asarray(input, dtype=np.float32)
    g = np.asarray(param, dtype=np.float32)
    if n <= 0:
        return inp.copy()
    nh = 8 // inp.shape[0]
    cfg = Cfg(
        Hout=inp.shape[2] // nh,
        Wout=inp.shape[3],
        C=inp.shape[1],
        P=g.shape[1],
        numIter=n,
        replicate_bottom=(nh == 1),
    )
    return _shard_and_run(inp, g, cfg)



# revision 9
# speedup vs baseline: 1.6474x; 1.6474x over previous
"""A-trous cross-bilateral filter (5x5 B3-spline stencil, numIter dilated passes)
on 8 TRN2 NeuronCores.

Sharding: host-side. The full [B,C,H,W] problem is cut into 8 overlapping
shards (batch x H-half, each with a `pad0 = 2*(2^n - 1)` halo taken from the
edge-padded full image), so each core computes its output slice with zero
cross-core communication.

Per-core kernel: for each iteration (dilation d = 1,2,4,8), stream row blocks
through SBUF. All 25 stencil shifts are plain AP offsets inside a
[rows, ch, W] tile. Per unique tap (12 after point symmetry):
  sq   = (g - g_shift)^2           (DVE, all 4 guidance channels packed)
  D    = sum_p sq                  (TensorE: 4 identity-matmul accumulates
                                    into PSUM, or DVE adds)
  w    = exp(-D + ln K2)           (ScalarE, bias folds the kernel coeff)
then the tap and its mirror accumulate w*x (3 channels) and w (denominator)
into PSUM via identity-matmul accumulation. The center tap is a scaled
identity matmul. Finally out = num * recip(den) and the block is stored to a
DRAM intermediate (bf16) for the next iteration.
"""

import math
from contextlib import ExitStack
from dataclasses import dataclass

import numpy as np

_K1 = np.array([1.0 / 16, 1.0 / 4, 3.0 / 8, 1.0 / 4, 1.0 / 16], dtype=np.float64)
_K2 = np.outer(_K1, _K1)  # [5,5]


def _pads(n):
    # pad remaining before iteration k (k=0..n); consumed 2*d per side per pass
    return [2 * ((1 << n) - (1 << k)) for k in range(n + 1)]


@dataclass
class Cfg:
    Hout: int = 512          # per-core output rows
    Wout: int = 1024         # output cols (not sharded)
    C: int = 3
    P: int = 4
    numIter: int = 4
    dt_elem: str = "bfloat16"    # elementwise/storage dtype: "bfloat16"|"float32"
    sqdiff: str = "tt"           # "tt" (sub+mult) | "custom" (fused DVE op) | "act" (sub + ACT square)
    d_reduce: str = "pe"         # "pe" | "dve"
    recip: str = "fast"          # "fast" | "exact"
    bcast_prod: bool = True      # w*x products via partition-bcast-free 3ch packed op
    reps: int = 1                # repeat whole pipeline (timing)
    # Reference semantics re-apply edge padding to the intermediate x each
    # iteration. Shards are arranged so the global row-edge is the TOP of
    # every shard (bottom halves are flipped on the host); between iterations
    # we re-replicate the top row band and both col bands of the DRAM
    # intermediate. replicate_bottom is for single-shard (full-image) tests.
    replicate_bottom: bool = False
    tap_filter: tuple = ()   # debug: restrict to these (dy,dx) unique taps
    report: bool = False     # print analytic per-engine busy estimate
    init: str = "pe"         # accumulator init: "pe" (scaled-identity matmul) | "dve"
    sqdiff_perf: bool = False  # enable 2x perf-mode slots for the custom sqdiff
    sq_act: int = 12         # first N unique taps use sub+ACT-square instead of sqdiff mode
                             # (HW-measured: all 12 on ScalarE beats any DVE mix)
    greedy_chunk: bool = False  # W chunks of max size instead of even split
    gp_prod: int = 0         # this many mirror-product ops per chunk go to GPSIMD
    rmax_slack: int = 2      # Rmax = 128 - rmax_slack*d (2d covers the max row shift)
    pair_dy0: bool = True    # dy=0 taps: direct+mirror products in one DVE op
    fast_final: bool = True  # bf16 iters: ACT-copy num out of PSUM so the
                             # final num*recip(den) mult runs in DVE 2x mode
    # cache exp(-D) fields: iter k's even-offset taps are iter k+1's inner
    # 3x3, and g is constant across iterations, so those 4 unique weight
    # fields can be stored to DRAM and re-loaded instead of recomputed.
    w_cache: bool = False
    io_bufs: int = 2
    work_bufs: int = 3
    psum_d_bufs: int = 3


class _Est:
    """Analytic per-engine busy estimate (ns), from the documented TRN2
    cost formulas. DVE: (58+FD/acc)/0.96; ACT: (224+FD)/1.2;
    PE: max(60, 6+FD)/2.4; DMA: bytes at ~185 GB/s/core aggregate."""

    def __init__(self):
        self.ns = {"DVE": 0.0, "ACT": 0.0, "PE": 0.0, "DMA": 0.0}
        self.cnt = {"DVE": 0, "ACT": 0, "PE": 0, "DMA": 0}

    def dve(self, fd, acc=1, n=1):
        self.ns["DVE"] += n * (58 + fd / acc) / 0.96
        self.cnt["DVE"] += n

    def act(self, fd, n=1):
        self.ns["ACT"] += n * (224 + fd) / 1.2
        self.cnt["ACT"] += n

    def pe(self, fd, n=1):
        self.ns["PE"] += n * max(60, 6 + fd) / 2.4
        self.cnt["PE"] += n

    def dma(self, nbytes, n=1):
        self.ns["DMA"] += nbytes / 185.0
        self.cnt["DMA"] += n

    def show(self, label):
        parts = ", ".join(
            f"{k}={self.ns[k]/1e3:.0f}us/{self.cnt[k]}" for k in self.ns
        )
        print(f"[est {label}] {parts}", flush=True)


def get_sqdiff_op(perf: bool = False):
    """Register a fused (a-b)^2 custom DVE op; sha pinned at first use."""
    from concourse import dve_ops
    from concourse.dve_spec import Spec, Src0, Src1, sq, lower, _has_src1
    from concourse.dve_uop import DveOpSpec

    name = "SQDIFF2X_ANT" if perf else "SQDIFF_ANT"
    for op in dve_ops.OPS:
        if op.name == name:
            return op
    spec = Spec(
        body=sq(Src0 - Src1),
        reference=lambda in0, in1, c0, c1, c2: (
            in0.astype(np.float32) - in1.astype(np.float32)
        )
        ** 2,
    )
    row = dve_ops._CUSTOM_DVE_ROW_BASE + len(dve_ops.OPS)
    assert row < 0x20
    shas = {}
    for ver in ("v3", "v4"):
        compiled = DveOpSpec(
            name=name, opcode=row, uops=lower(spec, ver=ver), rd1_en=_has_src1(spec)
        )
        shas[ver] = compiled.sha(ver)
    perf_en = {"v3": True, "v4": True} if perf else {}
    op = dve_ops.DveOp(name, spec, subdim=False, uops_sha=shas, perf_en=perf_en)
    dve_ops.OPS.append(op)
    dve_ops.CUSTOM_DVE_SPECS[name] = spec
    dve_ops._SUB_OPCODE_FOR_NAME[name] = row
    return op


def build(cfg: Cfg):
    """Build the per-core bass graph. Inputs 'x' [C,H0,W0], 'g' [P,H0,W0]
    (dt_elem), output 'out' [C,Hout,Wout] (f32)."""
    import concourse.bass as bass
    import concourse.tile as tile
    from concourse import bacc, mybir
    from concourse.masks import make_identity

    C, P, n = cfg.C, cfg.P, cfg.numIter
    assert 1 <= n <= 5
    pads = _pads(n)
    H = [cfg.Hout + 2 * p for p in pads]
    W = [cfg.Wout + 2 * p for p in pads]
    dt = getattr(mybir.dt, cfg.dt_elem)
    f32 = mybir.dt.float32
    AF = mybir.ActivationFunctionType
    OP = mybir.AluOpType

    sq_op = (
        get_sqdiff_op(cfg.sqdiff_perf)
        if cfg.sqdiff == "custom"
        else None
    )

    est = _Est()
    esz = 2 if cfg.dt_elem == "bfloat16" else 4
    acc2 = 2 if cfg.dt_elem == "bfloat16" else 1
    nc = bacc.Bacc("TRN2", target_bir_lowering=False, debug=False, num_devices=8)
    x_ext = nc.declare_dram_parameter("x", [C, H[0], W[0]], dt, isOutput=False)
    g_ext = nc.declare_dram_parameter("g", [P, H[0], W[0]], dt, isOutput=False)
    out_ext = nc.declare_dram_parameter("out", [C, cfg.Hout, cfg.Wout], f32, isOutput=True)

    # max extension of the weight field beyond a W-chunk: 2*d_max each side
    d_max = 1 << (n - 1)
    WE_MAX = 512  # psum bank limit for D / chunk accumulators

    with tile.TileContext(nc) as tc, ExitStack() as ctx:
        consts = ctx.enter_context(tc.tile_pool(name="consts", bufs=1))
        io = ctx.enter_context(tc.tile_pool(name="io", bufs=cfg.io_bufs))
        work = ctx.enter_context(tc.tile_pool(name="work", bufs=cfg.work_bufs))
        psum_acc = ctx.enter_context(tc.tile_pool(name="psum_acc", bufs=1, space="PSUM"))
        psum_d = ctx.enter_context(tc.tile_pool(name="psum_d", bufs=cfg.psum_d_bufs, space="PSUM"))
        dram = ctx.enter_context(tc.tile_pool(name="dram", bufs=1, space="DRAM"))

        # scaled shifted identities: sid(s, v)[k, m] = v iff k == m + s.
        # matmul(out, lhsT=sid(s, v), rhs) computes out[m] += v * rhs[m+s]:
        # the TensorEngine does both the partition (row) shift (compute
        # engines can only address SBUF operands at base partition 0/32/64/96)
        # and the K2 coefficient scaling for free.
        _sid = {}

        def sid(s, v):
            key = (s, round(v * 256))
            if key not in _sid:
                i = len(_sid)
                t = consts.tile([128, 128], dt, tag=f"sid{i}", name=f"sid{i}")
                nc.gpsimd.memset(t, 0.0)
                nc.gpsimd.affine_select(
                    out=t,
                    in_=t,
                    compare_op=mybir.AluOpType.not_equal,
                    fill=float(v),
                    base=-s,
                    pattern=[[-1, 128]],
                    channel_multiplier=1,
                )
                _sid[key] = t
            return _sid[key]

        ident = sid(0, 1.0)
        # pre-create every (shift, scale) combination used by the tap loops
        # so no const-tile setup ops land mid-pipeline.
        sid(0, float(_K2[2, 2]))
        for it0 in range(n):
            d0 = 1 << it0
            for dy in range(0, 3):
                for dx in range(-2, 3):
                    if dy > 0 or dx > 0:
                        v0 = float(_K2[2 + dy, 2 + dx])
                        sid(dy * d0, v0)
                        sid(0, v0)
        ones = consts.tile([128, WE_MAX], dt, tag="ones")
        nc.vector.memset(ones, 1.0)

        inter = [
            dram.tile([C, H[k], W[k]], dt, tag=f"inter{k}", name=f"inter{k}")
            for k in range(1, n)
        ]

        # exp(-D) field cache: iter it's even-offset unique taps are iter
        # it+1's inner-3x3 unique taps (same offsets, g fixed). Arrays are
        # indexed on iter-it's output grid with a (-2d, -2d) origin shift so
        # all stores/loads keep SBUF base partition 0.
        EVENS = ((0, 2), (2, 0), (2, 2), (2, -2))
        REUSE = {(0, 1): (0, 2), (1, 0): (2, 0), (1, 1): (2, 2), (1, -1): (2, -2)}
        ecache = {}
        if cfg.w_cache:
            for k in range(n - 1):
                dk = 1 << k
                for tkey in EVENS:
                    ecache[(k, tkey)] = dram.tile(
                        [2 * dk + H[k + 1], 4 * dk + W[k + 1]],
                        dt,
                        tag=f"ec{k}_{tkey[0]}_{tkey[1]}",
                        name=f"ec{k}_{tkey[0]}_{tkey[1]}".replace("-", "m"),
                    )

        for _rep in range(cfg.reps):
            for it in range(n):
                d = 1 << it
                src = x_ext if it == 0 else inter[it - 1]
                dst = out_ext if it == n - 1 else inter[it]
                Ws = W[it]
                Ho, Wo = H[it + 1], W[it + 1]
                gofs = pads[0] - pads[it]
                dt_out = f32 if it == n - 1 else dt

                Rmax = 128 - cfg.rmax_slack * d
                nb = math.ceil(Ho / Rmax)
                Rblk = math.ceil(Ho / nb)
                Wc_max = WE_MAX - 2 * d
                ncw = math.ceil(Wo / Wc_max)
                Wchunk = Wc_max if cfg.greedy_chunk else math.ceil(Wo / ncw)

                taps = [
                    (dy, dx)
                    for dy in range(0, 3)
                    for dx in range(-2, 3)
                    if (dy > 0 or dx > 0)
                ]
                if d == 1:
                    # odd-dx taps first: their shifted bf16 operands are
                    # 2B-misaligned (DVE falls to 1x), so let sq_act route
                    # them to the ScalarE square path.
                    taps.sort(key=lambda t: abs(t[1]) % 2 == 0)
                if cfg.tap_filter:
                    taps = [t for t in taps if t in cfg.tap_filter]

                for b in range(nb):
                    r0 = b * Rblk
                    R = min(Rblk, Ho - r0)
                    # Row-aligned views (tile row p <-> output row p - base):
                    #   x0/g0 base 0, xm1/g1 base -d, xm2/g2 base -2d.
                    # src row of output row q is q + 2d; g row adds gofs.
                    x0 = io.tile([128, C, Ws], dt, tag="x0")
                    xm1 = io.tile([128, C, Ws], dt, tag="xm1")
                    xm2 = io.tile([128, C, Ws], dt, tag="xm2")
                    g0 = io.tile([128, P, Ws], dt, tag="g0")
                    g1 = io.tile([128, P, Ws], dt, tag="g1")
                    g2 = io.tile([128, P, Ws], dt, tag="g2")
                    nc.sync.dma_start(
                        out=x0[: R + 2 * d],
                        in_=src[:, r0 + 2 * d : r0 + 3 * d + R + d, :].rearrange(
                            "c r w -> r c w"
                        ),
                    )
                    nc.sync.dma_start(
                        out=xm1[:R],
                        in_=src[:, r0 + d : r0 + d + R, :].rearrange("c r w -> r c w"),
                    )
                    nc.sync.dma_start(
                        out=xm2[:R],
                        in_=src[:, r0 : r0 + R, :].rearrange("c r w -> r c w"),
                    )
                    gr = gofs + r0
                    nc.sync.dma_start(
                        out=g0[: R + 2 * d],
                        in_=g_ext[
                            :, gr + 2 * d : gr + 2 * d + R + 2 * d, gofs : gofs + Ws
                        ].rearrange("c r w -> r c w"),
                    )
                    nc.sync.dma_start(
                        out=g1[: R + d],
                        in_=g_ext[
                            :, gr + d : gr + d + R + d, gofs : gofs + Ws
                        ].rearrange("c r w -> r c w"),
                    )
                    nc.sync.dma_start(
                        out=g2[: R + 2 * d],
                        in_=g_ext[
                            :, gr : gr + R + 2 * d, gofs : gofs + Ws
                        ].rearrange("c r w -> r c w"),
                    )
                    gdy = {0: g0, 1: g1, 2: g2}
                    xmdy = {0: x0, 1: xm1, 2: xm2}
                    est.dma(((3 * R + 2 * d) * C + (3 * R + 5 * d) * P) * Ws * esz, n=6)

                    for wci in range(ncw):
                        c0 = wci * Wchunk
                        Wc = min(Wchunk, Wo - c0)
                        num = psum_acc.tile([128, C, WE_MAX], f32, tag="num")
                        den = psum_acc.tile([128, WE_MAX], f32, tag="den")

                        # center tap initializes the accumulators:
                        # num = K2c * x, den = K2c
                        if cfg.init == "pe":
                            identC = sid(0, float(_K2[2, 2]))
                            for c in range(C):
                                nc.tensor.matmul(
                                    num[:R, c, :Wc],
                                    identC[:R, :R],
                                    x0[:R, c, 2 * d + c0 : 2 * d + c0 + Wc],
                                    start=True,
                                    stop=False,
                                    skip_group_check=True,
                                )
                            nc.tensor.matmul(
                                den[:R, :Wc],
                                identC[:R, :R],
                                ones[:R, :Wc],
                                start=True,
                                stop=False,
                                skip_group_check=True,
                            )
                            est.pe(Wc, n=4)
                        else:
                            nc.vector.tensor_scalar(
                                num[:R, :, :Wc],
                                x0[:R, :, 2 * d + c0 : 2 * d + c0 + Wc],
                                scalar1=float(_K2[2, 2]),
                                scalar2=None,
                                op0=OP.mult,
                            )
                            nc.vector.memset(den[:R, :Wc], float(_K2[2, 2]))
                            est.dve(C * Wc, acc=1)
                            est.dve(Wc, acc=2)

                        comp_i = 0
                        for ti, (dy, dx) in enumerate(taps):
                            last_tap = ti == len(taps) - 1
                            oy, ox = dy * d, dx * d
                            Re = R + oy
                            ce = c0 - max(ox, 0)
                            We = Wc + abs(ox)
                            k2v = float(_K2[2 + dy, 2 + dx])
                            # E field exp(-D) over rows [-oy, R), cols
                            # [ce, ce+We) (output coords); tile base row -oy.
                            # The K2 coefficient rides on the accumulation
                            # lhsT, so cached fields are coefficient-free.
                            wt = work.tile([128, WE_MAX + 2 * d_max * 2], dt, tag="wt")
                            reuse = (
                                cfg.w_cache and it > 0 and (dy, dx) in REUSE
                            )
                            if reuse:
                                ea = ecache[(it - 1, REUSE[(dy, dx)])]
                                nc.sync.dma_start(
                                    out=wt[:Re, :We],
                                    in_=ea[
                                        r0 - oy + 3 * d : r0 + R + 3 * d,
                                        ce + 3 * d : ce + We + 3 * d,
                                    ],
                                )
                                est.dma(Re * We * esz)
                            else:
                                gc = gdy[dy][:Re, :, 2 * d + ce : 2 * d + ce + We]
                                gs = g0[:Re, :, 2 * d + ce + ox : 2 * d + ce + ox + We]
                                sq_t = work.tile([128, P, WE_MAX + 2 * d_max * 2], dt, tag="sq")
                                sq_ap = sq_t[:Re, :, :We]
                                sq_mode = "act" if comp_i < cfg.sq_act else cfg.sqdiff
                                comp_i += 1
                                if sq_mode == "custom":
                                    nc.vector._custom_dve(sq_op, out=sq_ap, in0=gc, in1=gs)
                                    est.dve(P * We, acc=2 if cfg.sqdiff_perf else 1)
                                else:
                                    nc.vector.tensor_tensor(sq_ap, gc, gs, op=OP.subtract)
                                    est.dve(P * We, acc=acc2)
                                    if sq_mode == "act":
                                        nc.scalar.activation(sq_ap, sq_ap, AF.Square)
                                        est.act(P * We)
                                    else:
                                        nc.vector.tensor_tensor(sq_ap, sq_ap, sq_ap, op=OP.mult)
                                        est.dve(P * We, acc=acc2)

                                if cfg.d_reduce == "pe":
                                    Dp = psum_d.tile([128, WE_MAX], f32, tag="D")
                                    for p in range(P):
                                        nc.tensor.matmul(
                                            Dp[:Re, :We],
                                            ident[:Re, :Re],
                                            sq_t[:Re, p, :We],
                                            start=(p == 0),
                                            stop=(p == P - 1),
                                        )
                                    d_ap = Dp[:Re, :We]
                                    est.pe(We, n=4)
                                else:
                                    ds = work.tile([128, WE_MAX + 2 * d_max * 2], f32, tag="ds")
                                    nc.vector.tensor_tensor(
                                        ds[:Re, :We], sq_t[:Re, 0, :We], sq_t[:Re, 1, :We], op=OP.add
                                    )
                                    nc.vector.tensor_tensor(
                                        sq_t[:Re, 2, :We], sq_t[:Re, 2, :We], sq_t[:Re, 3, :We], op=OP.add
                                    )
                                    nc.vector.tensor_tensor(
                                        ds[:Re, :We], ds[:Re, :We], sq_t[:Re, 2, :We], op=OP.add
                                    )
                                    d_ap = ds[:Re, :We]
                                    est.dve(We, acc=acc2, n=3)

                                nc.scalar.activation(
                                    wt[:Re, :We], d_ap, AF.Exp, scale=-1.0,
                                )
                                est.act(We)
                                if cfg.w_cache and it < n - 1 and (dy, dx) in EVENS:
                                    ea = ecache[(it, (dy, dx))]
                                    nc.sync.dma_start(
                                        out=ea[
                                            r0 - oy + 2 * d : r0 + R + 2 * d,
                                            ce + 2 * d : ce + We + 2 * d,
                                        ],
                                        in_=wt[:Re, :We],
                                    )
                                    est.dma(Re * We * esz)

                            # direct tap (+oy,+ox): num[q] += w[q] * x[q+t].
                            # products in the shifted frame p = q + oy:
                            #   prod_d[p, v] = wt[p, c0+v] * x0[p, c0+v+ox]
                            # then the shifted-identity matmul folds rows
                            # p = m + oy into output row m.
                            w_d = wt[:Re, c0 - ce : c0 - ce + Wc]
                            x_d = x0[:Re, :, 2 * d + c0 + ox : 2 * d + c0 + ox + Wc]
                            # mirror tap (-oy,-ox): num[q] += w[q-t] * x[q-t],
                            # already base-aligned: wt row q <-> w[q - oy].
                            w_m = wt[:R, c0 - ox - ce : c0 - ox - ce + Wc]
                            x_m = xmdy[dy][:R, :, 2 * d + c0 - ox : 2 * d + c0 - ox + Wc]
                            prod = work.tile([128, 2, C, WE_MAX], dt, tag="prod")
                            if cfg.pair_dy0 and dy == 0 and ti >= cfg.gp_prod:
                                # direct and mirror share rows (Re == R) and
                                # tensors (x0, wt); fold both into one DVE op
                                # with a j dim: w cols step -ox, x cols -2*ox.
                                wj = bass.AP(
                                    tensor=w_d.tensor,
                                    offset=w_d.offset,
                                    ap=[w_d.ap[0], [-ox, 2], [0, C], w_d.ap[1]],
                                )
                                xj = bass.AP(
                                    tensor=x_d.tensor,
                                    offset=x_d.offset,
                                    ap=[x_d.ap[0], [-2 * ox, 2], x_d.ap[1], x_d.ap[2]],
                                )
                                nc.vector.tensor_tensor(
                                    prod[:R, :, :, :Wc], wj, xj, op=OP.mult
                                )
                                est.dve(2 * C * Wc, acc=acc2)
                                for j, (wv, sh, rr) in enumerate(
                                    ((w_d, 0, R), (w_m, 0, R))
                                ):
                                    stop = last_tap and j == 1
                                    lhsT = sid(0, k2v)
                                    for c in range(C):
                                        nc.tensor.matmul(
                                            num[:R, c, :Wc],
                                            lhsT[:rr, :R],
                                            prod[:rr, j, c, :Wc],
                                            start=False,
                                            stop=stop,
                                            skip_group_check=True,
                                        )
                                    nc.tensor.matmul(
                                        den[:R, :Wc],
                                        lhsT[:rr, :R],
                                        wv,
                                        start=False,
                                        stop=stop,
                                        skip_group_check=True,
                                    )
                                    est.pe(Wc, n=4)
                                continue
                            for j, (wv, xv, rr) in enumerate(
                                ((w_d, x_d, Re), (w_m, x_m, R))
                            ):
                                if j == 1 and ti < cfg.gp_prod:
                                    wb = bass.AP(
                                        tensor=wv.tensor,
                                        offset=wv.offset,
                                        ap=[wv.ap[0], [0, C], wv.ap[1]],
                                    )
                                    nc.gpsimd.tensor_tensor(
                                        prod[:rr, j, :, :Wc], wb, xv, op=OP.mult
                                    )
                                    continue
                                if cfg.bcast_prod:
                                    wb = bass.AP(
                                        tensor=wv.tensor,
                                        offset=wv.offset,
                                        ap=[wv.ap[0], [0, C], wv.ap[1]],
                                    )
                                    nc.vector.tensor_tensor(
                                        prod[:rr, j, :, :Wc], wb, xv, op=OP.mult
                                    )
                                    est.dve(C * Wc, acc=acc2)
                                else:
                                    for c in range(C):
                                        nc.vector.tensor_tensor(
                                            prod[:rr, j, c, :Wc], wv, xv[:, c, :], op=OP.mult
                                        )
                                        est.dve(Wc, acc=acc2)
                            for j, (wv, sh, rr) in enumerate(
                                ((w_d, oy, Re), (w_m, 0, R))
                            ):
                                stop = last_tap and j == 1
                                lhsT = sid(sh, k2v)
                                for c in range(C):
                                    nc.tensor.matmul(
                                        num[:R, c, :Wc],
                                        lhsT[:rr, :R],
                                        prod[:rr, j, c, :Wc],
                                        start=False,
                                        stop=stop,
                                        skip_group_check=True,
                                    )
                                nc.tensor.matmul(
                                    den[:R, :Wc],
                                    lhsT[:rr, :R],
                                    wv,
                                    start=False,
                                    stop=stop,
                                    skip_group_check=True,
                                )
                                est.pe(Wc, n=4)

                        rden = work.tile([128, WE_MAX], f32, tag="rden")
                        if cfg.recip == "fast":
                            nc.vector.reciprocal_approx_fast(rden[:R, :Wc], den[:R, :Wc])
                        else:
                            nc.vector.reciprocal(rden[:R, :Wc], den[:R, :Wc])
                        est.dve(Wc, acc=1)
                        outt = io.tile([128, C, WE_MAX], dt_out, tag="outt")
                        if cfg.fast_final and dt_out != f32:
                            # hoist the f32->bf16 rounds onto ScalarE so the
                            # final mult has all-SBUF 2-byte operands (DVE 2x)
                            rden_b = work.tile([128, WE_MAX], dt, tag="rdenb")
                            nc.scalar.activation(
                                rden_b[:R, :Wc], rden[:R, :Wc], AF.Identity
                            )
                            numb = work.tile([128, C, WE_MAX], dt, tag="numb")
                            nc.scalar.activation(
                                numb[:R, :, :Wc], num[:R, :, :Wc], AF.Identity
                            )
                            est.act(Wc)
                            est.act(C * Wc)
                            rsl = rden_b[:R, :Wc]
                            rb = bass.AP(
                                tensor=rsl.tensor,
                                offset=rsl.offset,
                                ap=[list(rsl.ap[0]), [0, C], list(rsl.ap[1])],
                            )
                            nc.vector.tensor_tensor(
                                outt[:R, :, :Wc], numb[:R, :, :Wc], rb, op=OP.mult
                            )
                            est.dve(C * Wc, acc=acc2)
                        else:
                            rsl = rden[:R, :Wc]
                            rb = bass.AP(
                                tensor=rsl.tensor,
                                offset=rsl.offset,
                                ap=[list(rsl.ap[0]), [0, C], list(rsl.ap[1])],
                            )
                            nc.vector.tensor_tensor(
                                outt[:R, :, :Wc], num[:R, :, :Wc], rb, op=OP.mult
                            )
                            est.dve(C * Wc, acc=1)
                        # reference semantics: out-of-image cols of the
                        # intermediate are edge replicas. Overwrite the bands
                        # in-tile (bias-broadcast copy) before storing.
                        if it < n - 1 and pads[it + 1] > 0:
                            pn = pads[it + 1]
                            if wci == 0:
                                for c in range(C):
                                    nc.scalar.activation(
                                        outt[:R, c, 0:pn], outt[:R, c, 0:pn],
                                        AF.Identity, scale=0.0,
                                        bias=outt[:R, c, pn : pn + 1],
                                    )
                            if wci == ncw - 1:
                                for c in range(C):
                                    nc.scalar.activation(
                                        outt[:R, c, Wc - pn : Wc], outt[:R, c, Wc - pn : Wc],
                                        AF.Identity, scale=0.0,
                                        bias=outt[:R, c, Wc - pn - 1 : Wc - pn],
                                    )
                        nc.sync.dma_start(
                            out=dst[:, r0 : r0 + R, c0 : c0 + Wc].rearrange(
                                "c r w -> r c w"
                            ),
                            in_=outt[:R, :, :Wc],
                        )
                        est.dma(R * C * Wc * (4 if it == n - 1 else esz))

                # reference semantics: out-of-image rows of the intermediate
                # are edge replicas (row-broadcast DMA; innermost dim stays
                # contiguous so DGE accepts it). Cols were fixed in-tile.
                if it < n - 1:
                    pk = pads[it + 1]
                    if pk > 0:
                        def _bcast(src_ap, axis_idx, count):
                            ap = [list(a) for a in src_ap.ap]
                            ap[axis_idx] = [0, count]
                            return bass.AP(tensor=src_ap.tensor, offset=src_ap.offset, ap=ap)

                        # top rows [0, pk) := row pk
                        nc.sync.dma_start(
                            out=dst[:, 0:pk, :],
                            in_=_bcast(dst[:, pk : pk + 1, :], 1, pk),
                        )
                        if cfg.replicate_bottom:
                            nc.sync.dma_start(
                                out=dst[:, Ho - pk : Ho, :],
                                in_=_bcast(dst[:, Ho - pk - 1 : Ho - pk, :], 1, pk),
                            )

    if cfg.report:
        est.show(f"n={n} dt={cfg.dt_elem} sqdiff={cfg.sqdiff} d_reduce={cfg.d_reduce} reps={cfg.reps}")
    nc.compile()
    return nc


# ---------------------------------------------------------------------------
# host side
# ---------------------------------------------------------------------------


def _shard_and_run(inp: np.ndarray, param: np.ndarray, cfg: Cfg):
    import ml_dtypes
    from concourse.bass_utils import run_bass_kernel_spmd

    B, C, Hfull, Wfull = inp.shape
    n = cfg.numIter
    pad0 = _pads(n)[0]
    nh = 8 // B
    assert nh in (1, 2), "sharding assumes each shard has at most one global row-edge"
    Hsh = Hfull // nh
    assert Hsh == cfg.Hout and Wfull == cfg.Wout

    np_dt = np.float32 if cfg.dt_elem == "float32" else ml_dtypes.bfloat16
    xp = np.pad(inp, ((0, 0), (0, 0), (pad0, pad0), (pad0, pad0)), mode="edge").astype(np_dt)
    gp = np.pad(param, ((0, 0), (0, 0), (pad0, pad0), (pad0, pad0)), mode="edge").astype(np_dt)

    in_maps = []
    for b in range(B):
        for h in range(nh):
            r = h * Hsh
            xs = xp[b, :, r : r + Hsh + 2 * pad0, :]
            gs = gp[b, :, r : r + Hsh + 2 * pad0, :]
            if h == nh - 1 and nh > 1:
                # flip so the global row-edge is at the top of the shard
                xs, gs = xs[:, ::-1, :], gs[:, ::-1, :]
            in_maps.append(
                {"x": np.ascontiguousarray(xs), "g": np.ascontiguousarray(gs)}
            )

    nc = build(cfg)
    res = run_bass_kernel_spmd(nc, in_maps, core_ids=list(range(8)))

    out = np.empty((B, C, Hfull, Wfull), dtype=np.float32)
    for b in range(B):
        for h in range(nh):
            o = res.results[b * nh + h]["out"]
            if h == nh - 1 and nh > 1:
                o = o[:, ::-1, :]
            out[b, :, h * Hsh : (h + 1) * Hsh, :] = o
    return out


def kernel(input: np.ndarray, param: np.ndarray, numIter) -> np.ndarray:
    n = int(numIter)
    inp = np.asarray(input, dtype=np.float32)
    g = np.asarray(param, dtype=np.float32)
    if n <= 0:
        return inp.copy()
    nh = 8 // inp.shape[0]
    cfg = Cfg(
        Hout=inp.shape[2] // nh,
        Wout=inp.shape[3],
        C=inp.shape[1],
        P=g.shape[1],
        numIter=n,
        replicate_bottom=(nh == 1),
    )
    return _shard_and_run(inp, g, cfg)



# revision 13
# speedup vs baseline: 1.7972x; 1.0910x over previous
"""A-trous cross-bilateral filter (5x5 B3-spline stencil, numIter dilated passes)
on 8 TRN2 NeuronCores.

Sharding: host-side. The full [B,C,H,W] problem is cut into 8 overlapping
shards (batch x H-half, each with a `pad0 = 2*(2^n - 1)` halo taken from the
edge-padded full image), so each core computes its output slice with zero
cross-core communication.

Per-core kernel: for each iteration (dilation d = 1,2,4,8), stream row blocks
through SBUF. All 25 stencil shifts are plain AP offsets inside a
[rows, ch, W] tile. Per unique tap (12 after point symmetry):
  sq   = (g - g_shift)^2           (DVE, all 4 guidance channels packed)
  D    = sum_p sq                  (TensorE: 4 identity-matmul accumulates
                                    into PSUM, or DVE adds)
  w    = exp(-D + ln K2)           (ScalarE, bias folds the kernel coeff)
then the tap and its mirror accumulate w*x (3 channels) and w (denominator)
into PSUM via identity-matmul accumulation. The center tap is a scaled
identity matmul. Finally out = num * recip(den) and the block is stored to a
DRAM intermediate (bf16) for the next iteration.
"""

import math
from contextlib import ExitStack
from dataclasses import dataclass

import numpy as np

_K1 = np.array([1.0 / 16, 1.0 / 4, 3.0 / 8, 1.0 / 4, 1.0 / 16], dtype=np.float64)
_K2 = np.outer(_K1, _K1)  # [5,5]


def _pads(n):
    # pad remaining before iteration k (k=0..n); consumed 2*d per side per pass
    return [2 * ((1 << n) - (1 << k)) for k in range(n + 1)]


@dataclass
class Cfg:
    Hout: int = 512          # per-core output rows
    Wout: int = 1024         # output cols (not sharded)
    C: int = 3
    P: int = 4
    numIter: int = 4
    dt_elem: str = "bfloat16"    # elementwise/storage dtype: "bfloat16"|"float32"
    sqdiff: str = "tt"           # "tt" (sub+mult) | "custom" (fused DVE op) | "act" (sub + ACT square)
    d_reduce: str = "pe"         # "pe" | "dve"
    recip: str = "fast"          # "fast" | "exact"
    bcast_prod: bool = True      # w*x products via partition-bcast-free 3ch packed op
    reps: int = 1                # repeat whole pipeline (timing)
    # Reference semantics re-apply edge padding to the intermediate x each
    # iteration. Shards are arranged so the global row-edge is the TOP of
    # every shard (bottom halves are flipped on the host); between iterations
    # we re-replicate the top row band and both col bands of the DRAM
    # intermediate. replicate_bottom is for single-shard (full-image) tests.
    replicate_bottom: bool = False
    tap_filter: tuple = ()   # debug: restrict to these (dy,dx) unique taps
    report: bool = False     # print analytic per-engine busy estimate
    init: str = "pe"         # accumulator init: "pe" (scaled-identity matmul) | "dve"
    sqdiff_perf: bool = False  # enable 2x perf-mode slots for the custom sqdiff
    sq_act: int = 12         # first N unique taps use sub+ACT-square instead of sqdiff mode
                             # (HW-measured: all 12 on ScalarE beats any DVE mix)
    greedy_chunk: bool = False  # W chunks of max size instead of even split
    gp_prod: int = 0         # this many mirror-product ops per chunk go to GPSIMD
    rmax_slack: int = 2      # Rmax = 128 - rmax_slack*d (2d covers the max row shift)
    pair_dy0: bool = True    # dy=0 taps: direct+mirror products in one DVE op
    fast_final: bool = True  # bf16 iters: ACT-copy num out of PSUM so the
                             # final num*recip(den) mult runs in DVE 2x mode
    pack_mm: bool = False    # one matmul per shift for all C num channels
                             # (multi-dim rhs/out free AP; 3x fewer PE instrs)
    # cache exp(-D) fields: iter k's even-offset taps are iter k+1's inner
    # 3x3, and g is constant across iterations, so those 4 unique weight
    # fields can be stored to DRAM and re-loaded instead of recomputed.
    w_cache: bool = False
    io_bufs: int = 2
    work_bufs: int = 3
    psum_d_bufs: int = 3


class _Est:
    """Analytic per-engine busy estimate (ns), from the documented TRN2
    cost formulas. DVE: (58+FD/acc)/0.96; ACT: (224+FD)/1.2;
    PE: max(60, 6+FD)/2.4; DMA: bytes at ~185 GB/s/core aggregate."""

    def __init__(self):
        self.ns = {"DVE": 0.0, "ACT": 0.0, "PE": 0.0, "DMA": 0.0}
        self.cnt = {"DVE": 0, "ACT": 0, "PE": 0, "DMA": 0}

    def dve(self, fd, acc=1, n=1):
        self.ns["DVE"] += n * (58 + fd / acc) / 0.96
        self.cnt["DVE"] += n

    def act(self, fd, n=1):
        self.ns["ACT"] += n * (224 + fd) / 1.2
        self.cnt["ACT"] += n

    def pe(self, fd, n=1):
        self.ns["PE"] += n * max(60, 6 + fd) / 2.4
        self.cnt["PE"] += n

    def dma(self, nbytes, n=1):
        self.ns["DMA"] += nbytes / 185.0
        self.cnt["DMA"] += n

    def show(self, label):
        parts = ", ".join(
            f"{k}={self.ns[k]/1e3:.0f}us/{self.cnt[k]}" for k in self.ns
        )
        print(f"[est {label}] {parts}", flush=True)


def get_sqdiff_op(perf: bool = False):
    """Register a fused (a-b)^2 custom DVE op; sha pinned at first use."""
    from concourse import dve_ops
    from concourse.dve_spec import Spec, Src0, Src1, sq, lower, _has_src1
    from concourse.dve_uop import DveOpSpec

    name = "SQDIFF2X_ANT" if perf else "SQDIFF_ANT"
    for op in dve_ops.OPS:
        if op.name == name:
            return op
    spec = Spec(
        body=sq(Src0 - Src1),
        reference=lambda in0, in1, c0, c1, c2: (
            in0.astype(np.float32) - in1.astype(np.float32)
        )
        ** 2,
    )
    row = dve_ops._CUSTOM_DVE_ROW_BASE + len(dve_ops.OPS)
    assert row < 0x20
    shas = {}
    for ver in ("v3", "v4"):
        compiled = DveOpSpec(
            name=name, opcode=row, uops=lower(spec, ver=ver), rd1_en=_has_src1(spec)
        )
        shas[ver] = compiled.sha(ver)
    perf_en = {"v3": True, "v4": True} if perf else {}
    op = dve_ops.DveOp(name, spec, subdim=False, uops_sha=shas, perf_en=perf_en)
    dve_ops.OPS.append(op)
    dve_ops.CUSTOM_DVE_SPECS[name] = spec
    dve_ops._SUB_OPCODE_FOR_NAME[name] = row
    return op


def build(cfg: Cfg):
    """Build the per-core bass graph. Inputs 'x' [C,H0,W0], 'g' [P,H0,W0]
    (dt_elem), output 'out' [C,Hout,Wout] (f32)."""
    import concourse.bass as bass
    import concourse.tile as tile
    from concourse import bacc, mybir
    from concourse.masks import make_identity

    C, P, n = cfg.C, cfg.P, cfg.numIter
    assert 1 <= n <= 5
    pads = _pads(n)
    H = [cfg.Hout + 2 * p for p in pads]
    W = [cfg.Wout + 2 * p for p in pads]
    dt = getattr(mybir.dt, cfg.dt_elem)
    f32 = mybir.dt.float32
    AF = mybir.ActivationFunctionType
    OP = mybir.AluOpType

    sq_op = (
        get_sqdiff_op(cfg.sqdiff_perf)
        if cfg.sqdiff == "custom"
        else None
    )

    est = _Est()
    esz = 2 if cfg.dt_elem == "bfloat16" else 4
    acc2 = 2 if cfg.dt_elem == "bfloat16" else 1
    nc = bacc.Bacc("TRN2", target_bir_lowering=False, debug=False, num_devices=8)
    x_ext = nc.declare_dram_parameter("x", [C, H[0], W[0]], dt, isOutput=False)
    g_ext = nc.declare_dram_parameter("g", [P, H[0], W[0]], dt, isOutput=False)
    out_ext = nc.declare_dram_parameter("out", [C, cfg.Hout, cfg.Wout], f32, isOutput=True)

    # max extension of the weight field beyond a W-chunk: 2*d_max each side
    d_max = 1 << (n - 1)
    WE_MAX = 512  # psum bank limit for D / chunk accumulators

    with tile.TileContext(nc) as tc, ExitStack() as ctx:
        consts = ctx.enter_context(tc.tile_pool(name="consts", bufs=1))
        io = ctx.enter_context(tc.tile_pool(name="io", bufs=cfg.io_bufs))
        work = ctx.enter_context(tc.tile_pool(name="work", bufs=cfg.work_bufs))
        psum_acc = ctx.enter_context(tc.tile_pool(name="psum_acc", bufs=1, space="PSUM"))
        psum_d = ctx.enter_context(tc.tile_pool(name="psum_d", bufs=cfg.psum_d_bufs, space="PSUM"))
        dram = ctx.enter_context(tc.tile_pool(name="dram", bufs=1, space="DRAM"))

        # scaled shifted identities: sid(s, v)[k, m] = v iff k == m + s.
        # matmul(out, lhsT=sid(s, v), rhs) computes out[m] += v * rhs[m+s]:
        # the TensorEngine does both the partition (row) shift (compute
        # engines can only address SBUF operands at base partition 0/32/64/96)
        # and the K2 coefficient scaling for free.
        _sid = {}

        def sid(s, v):
            key = (s, round(v * 256))
            if key not in _sid:
                i = len(_sid)
                t = consts.tile([128, 128], dt, tag=f"sid{i}", name=f"sid{i}")
                nc.gpsimd.memset(t, 0.0)
                nc.gpsimd.affine_select(
                    out=t,
                    in_=t,
                    compare_op=mybir.AluOpType.not_equal,
                    fill=float(v),
                    base=-s,
                    pattern=[[-1, 128]],
                    channel_multiplier=1,
                )
                _sid[key] = t
            return _sid[key]

        ident = sid(0, 1.0)
        # pre-create every (shift, scale) combination used by the tap loops
        # so no const-tile setup ops land mid-pipeline.
        sid(0, float(_K2[2, 2]))
        for it0 in range(n):
            d0 = 1 << it0
            for dy in range(0, 3):
                for dx in range(-2, 3):
                    if dy > 0 or dx > 0:
                        v0 = float(_K2[2 + dy, 2 + dx])
                        sid(dy * d0, v0)
                        sid(0, v0)
        ones = consts.tile([128, WE_MAX], dt, tag="ones")
        nc.vector.memset(ones, 1.0)

        inter = [
            dram.tile([C, H[k], W[k]], dt, tag=f"inter{k}", name=f"inter{k}")
            for k in range(1, n)
        ]

        # exp(-D) field cache: iter it's even-offset unique taps are iter
        # it+1's inner-3x3 unique taps (same offsets, g fixed). Arrays are
        # indexed on iter-it's output grid with a (-2d, -2d) origin shift so
        # all stores/loads keep SBUF base partition 0.
        EVENS = ((0, 2), (2, 0), (2, 2), (2, -2))
        REUSE = {(0, 1): (0, 2), (1, 0): (2, 0), (1, 1): (2, 2), (1, -1): (2, -2)}
        ecache = {}
        if cfg.w_cache:
            for k in range(n - 1):
                dk = 1 << k
                for tkey in EVENS:
                    ecache[(k, tkey)] = dram.tile(
                        [2 * dk + H[k + 1], 4 * dk + W[k + 1]],
                        dt,
                        tag=f"ec{k}_{tkey[0]}_{tkey[1]}",
                        name=f"ec{k}_{tkey[0]}_{tkey[1]}".replace("-", "m"),
                    )

        for _rep in range(cfg.reps):
            for it in range(n):
                d = 1 << it
                src = x_ext if it == 0 else inter[it - 1]
                dst = out_ext if it == n - 1 else inter[it]
                Ws = W[it]
                Ho, Wo = H[it + 1], W[it + 1]
                gofs = pads[0] - pads[it]
                dt_out = f32 if it == n - 1 else dt

                Rmax = 128 - cfg.rmax_slack * d
                nb = math.ceil(Ho / Rmax)
                Rblk = math.ceil(Ho / nb)
                Wc_max = WE_MAX - 2 * d
                ncw = math.ceil(Wo / Wc_max)
                Wchunk = Wc_max if cfg.greedy_chunk else math.ceil(Wo / ncw)

                taps = [
                    (dy, dx)
                    for dy in range(0, 3)
                    for dx in range(-2, 3)
                    if (dy > 0 or dx > 0)
                ]
                if d == 1:
                    # odd-dx taps first: their shifted bf16 operands are
                    # 2B-misaligned (DVE falls to 1x), so let sq_act route
                    # them to the ScalarE square path.
                    taps.sort(key=lambda t: abs(t[1]) % 2 == 0)
                if cfg.tap_filter:
                    taps = [t for t in taps if t in cfg.tap_filter]

                for b in range(nb):
                    r0 = b * Rblk
                    R = min(Rblk, Ho - r0)
                    # Row-aligned views (tile row p <-> output row p - base):
                    #   x0/g0 base 0, xm1/g1 base -d, xm2/g2 base -2d.
                    # src row of output row q is q + 2d; g row adds gofs.
                    x0 = io.tile([128, C, Ws], dt, tag="x0")
                    xm1 = io.tile([128, C, Ws], dt, tag="xm1")
                    xm2 = io.tile([128, C, Ws], dt, tag="xm2")
                    g0 = io.tile([128, P, Ws], dt, tag="g0")
                    g1 = io.tile([128, P, Ws], dt, tag="g1")
                    g2 = io.tile([128, P, Ws], dt, tag="g2")
                    nc.sync.dma_start(
                        out=x0[: R + 2 * d],
                        in_=src[:, r0 + 2 * d : r0 + 3 * d + R + d, :].rearrange(
                            "c r w -> r c w"
                        ),
                    )
                    nc.sync.dma_start(
                        out=xm1[:R],
                        in_=src[:, r0 + d : r0 + d + R, :].rearrange("c r w -> r c w"),
                    )
                    nc.sync.dma_start(
                        out=xm2[:R],
                        in_=src[:, r0 : r0 + R, :].rearrange("c r w -> r c w"),
                    )
                    gr = gofs + r0
                    nc.sync.dma_start(
                        out=g0[: R + 2 * d],
                        in_=g_ext[
                            :, gr + 2 * d : gr + 2 * d + R + 2 * d, gofs : gofs + Ws
                        ].rearrange("c r w -> r c w"),
                    )
                    nc.sync.dma_start(
                        out=g1[: R + d],
                        in_=g_ext[
                            :, gr + d : gr + d + R + d, gofs : gofs + Ws
                        ].rearrange("c r w -> r c w"),
                    )
                    nc.sync.dma_start(
                        out=g2[: R + 2 * d],
                        in_=g_ext[
                            :, gr : gr + R + 2 * d, gofs : gofs + Ws
                        ].rearrange("c r w -> r c w"),
                    )
                    gdy = {0: g0, 1: g1, 2: g2}
                    xmdy = {0: x0, 1: xm1, 2: xm2}
                    est.dma(((3 * R + 2 * d) * C + (3 * R + 5 * d) * P) * Ws * esz, n=6)

                    for wci in range(ncw):
                        c0 = wci * Wchunk
                        Wc = min(Wchunk, Wo - c0)
                        num = psum_acc.tile([128, C, WE_MAX], f32, tag="num")
                        den = psum_acc.tile([128, WE_MAX], f32, tag="den")

                        # center tap initializes the accumulators:
                        # num = K2c * x, den = K2c
                        if cfg.init == "pe":
                            identC = sid(0, float(_K2[2, 2]))
                            if cfg.pack_mm:
                                nc.tensor.matmul(
                                    num[:R, :, :Wc],
                                    identC[:R, :R],
                                    x0[:R, :, 2 * d + c0 : 2 * d + c0 + Wc],
                                    start=True,
                                    stop=False,
                                    skip_group_check=True,
                                )
                            else:
                                for c in range(C):
                                    nc.tensor.matmul(
                                        num[:R, c, :Wc],
                                        identC[:R, :R],
                                        x0[:R, c, 2 * d + c0 : 2 * d + c0 + Wc],
                                        start=True,
                                        stop=False,
                                        skip_group_check=True,
                                    )
                            nc.tensor.matmul(
                                den[:R, :Wc],
                                identC[:R, :R],
                                ones[:R, :Wc],
                                start=True,
                                stop=False,
                                skip_group_check=True,
                            )
                            est.pe(Wc, n=4)
                        else:
                            nc.vector.tensor_scalar(
                                num[:R, :, :Wc],
                                x0[:R, :, 2 * d + c0 : 2 * d + c0 + Wc],
                                scalar1=float(_K2[2, 2]),
                                scalar2=None,
                                op0=OP.mult,
                            )
                            nc.vector.memset(den[:R, :Wc], float(_K2[2, 2]))
                            est.dve(C * Wc, acc=1)
                            est.dve(Wc, acc=2)

                        comp_i = 0
                        for ti, (dy, dx) in enumerate(taps):
                            last_tap = ti == len(taps) - 1
                            oy, ox = dy * d, dx * d
                            Re = R + oy
                            ce = c0 - max(ox, 0)
                            We = Wc + abs(ox)
                            k2v = float(_K2[2 + dy, 2 + dx])
                            # E field exp(-D) over rows [-oy, R), cols
                            # [ce, ce+We) (output coords); tile base row -oy.
                            # The K2 coefficient rides on the accumulation
                            # lhsT, so cached fields are coefficient-free.
                            wt = work.tile([128, WE_MAX + 2 * d_max * 2], dt, tag="wt")
                            reuse = (
                                cfg.w_cache and it > 0 and (dy, dx) in REUSE
                            )
                            if reuse:
                                ea = ecache[(it - 1, REUSE[(dy, dx)])]
                                nc.sync.dma_start(
                                    out=wt[:Re, :We],
                                    in_=ea[
                                        r0 - oy + 3 * d : r0 + R + 3 * d,
                                        ce + 3 * d : ce + We + 3 * d,
                                    ],
                                )
                                est.dma(Re * We * esz)
                            else:
                                gc = gdy[dy][:Re, :, 2 * d + ce : 2 * d + ce + We]
                                gs = g0[:Re, :, 2 * d + ce + ox : 2 * d + ce + ox + We]
                                sq_t = work.tile([128, P, WE_MAX + 2 * d_max * 2], dt, tag="sq")
                                sq_ap = sq_t[:Re, :, :We]
                                sq_mode = "act" if comp_i < cfg.sq_act else cfg.sqdiff
                                comp_i += 1
                                if sq_mode == "custom":
                                    nc.vector._custom_dve(sq_op, out=sq_ap, in0=gc, in1=gs)
                                    est.dve(P * We, acc=2 if cfg.sqdiff_perf else 1)
                                else:
                                    nc.vector.tensor_tensor(sq_ap, gc, gs, op=OP.subtract)
                                    est.dve(P * We, acc=acc2)
                                    if sq_mode == "act":
                                        nc.scalar.activation(sq_ap, sq_ap, AF.Square)
                                        est.act(P * We)
                                    else:
                                        nc.vector.tensor_tensor(sq_ap, sq_ap, sq_ap, op=OP.mult)
                                        est.dve(P * We, acc=acc2)

                                if cfg.d_reduce == "pe":
                                    Dp = psum_d.tile([128, WE_MAX], f32, tag="D")
                                    for p in range(P):
                                        nc.tensor.matmul(
                                            Dp[:Re, :We],
                                            ident[:Re, :Re],
                                            sq_t[:Re, p, :We],
                                            start=(p == 0),
                                            stop=(p == P - 1),
                                        )
                                    d_ap = Dp[:Re, :We]
                                    est.pe(We, n=4)
                                else:
                                    ds = work.tile([128, WE_MAX + 2 * d_max * 2], f32, tag="ds")
                                    nc.vector.tensor_tensor(
                                        ds[:Re, :We], sq_t[:Re, 0, :We], sq_t[:Re, 1, :We], op=OP.add
                                    )
                                    nc.vector.tensor_tensor(
                                        sq_t[:Re, 2, :We], sq_t[:Re, 2, :We], sq_t[:Re, 3, :We], op=OP.add
                                    )
                                    nc.vector.tensor_tensor(
                                        ds[:Re, :We], ds[:Re, :We], sq_t[:Re, 2, :We], op=OP.add
                                    )
                                    d_ap = ds[:Re, :We]
                                    est.dve(We, acc=acc2, n=3)

                                nc.scalar.activation(
                                    wt[:Re, :We], d_ap, AF.Exp, scale=-1.0,
                                )
                                est.act(We)
                                if cfg.w_cache and it < n - 1 and (dy, dx) in EVENS:
                                    ea = ecache[(it, (dy, dx))]
                                    nc.sync.dma_start(
                                        out=ea[
                                            r0 - oy + 2 * d : r0 + R + 2 * d,
                                            ce + 2 * d : ce + We + 2 * d,
                                        ],
                                        in_=wt[:Re, :We],
                                    )
                                    est.dma(Re * We * esz)

                            # direct tap (+oy,+ox): num[q] += w[q] * x[q+t].
                            # products in the shifted frame p = q + oy:
                            #   prod_d[p, v] = wt[p, c0+v] * x0[p, c0+v+ox]
                            # then the shifted-identity matmul folds rows
                            # p = m + oy into output row m.
                            w_d = wt[:Re, c0 - ce : c0 - ce + Wc]
                            x_d = x0[:Re, :, 2 * d + c0 + ox : 2 * d + c0 + ox + Wc]
                            # mirror tap (-oy,-ox): num[q] += w[q-t] * x[q-t],
                            # already base-aligned: wt row q <-> w[q - oy].
                            w_m = wt[:R, c0 - ox - ce : c0 - ox - ce + Wc]
                            x_m = xmdy[dy][:R, :, 2 * d + c0 - ox : 2 * d + c0 - ox + Wc]
                            prod = work.tile([128, 2, C, WE_MAX], dt, tag="prod")
                            if cfg.pair_dy0 and dy == 0 and ti >= cfg.gp_prod:
                                # direct and mirror share rows (Re == R) and
                                # tensors (x0, wt); fold both into one DVE op
                                # with a j dim: w cols step -ox, x cols -2*ox.
                                wj = bass.AP(
                                    tensor=w_d.tensor,
                                    offset=w_d.offset,
                                    ap=[w_d.ap[0], [-ox, 2], [0, C], w_d.ap[1]],
                                )
                                xj = bass.AP(
                                    tensor=x_d.tensor,
                                    offset=x_d.offset,
                                    ap=[x_d.ap[0], [-2 * ox, 2], x_d.ap[1], x_d.ap[2]],
                                )
                                nc.vector.tensor_tensor(
                                    prod[:R, :, :, :Wc], wj, xj, op=OP.mult
                                )
                                est.dve(2 * C * Wc, acc=acc2)
                                for j, (wv, sh, rr) in enumerate(
                                    ((w_d, 0, R), (w_m, 0, R))
                                ):
                                    stop = last_tap and j == 1
                                    lhsT = sid(0, k2v)
                                    if cfg.pack_mm:
                                        nc.tensor.matmul(
                                            num[:R, :, :Wc],
                                            lhsT[:rr, :R],
                                            prod[:rr, j, :, :Wc],
                                            start=False,
                                            stop=stop,
                                            skip_group_check=True,
                                        )
                                    else:
                                        for c in range(C):
                                            nc.tensor.matmul(
                                                num[:R, c, :Wc],
                                                lhsT[:rr, :R],
                                                prod[:rr, j, c, :Wc],
                                                start=False,
                                                stop=stop,
                                                skip_group_check=True,
                                            )
                                    nc.tensor.matmul(
                                        den[:R, :Wc],
                                        lhsT[:rr, :R],
                                        wv,
                                        start=False,
                                        stop=stop,
                                        skip_group_check=True,
                                    )
                                    est.pe(Wc, n=4)
                                continue
                            for j, (wv, xv, rr) in enumerate(
                                ((w_d, x_d, Re), (w_m, x_m, R))
                            ):
                                if j == 1 and ti < cfg.gp_prod:
                                    wb = bass.AP(
                                        tensor=wv.tensor,
                                        offset=wv.offset,
                                        ap=[wv.ap[0], [0, C], wv.ap[1]],
                                    )
                                    nc.gpsimd.tensor_tensor(
                                        prod[:rr, j, :, :Wc], wb, xv, op=OP.mult
                                    )
                                    continue
                                if cfg.bcast_prod:
                                    wb = bass.AP(
                                        tensor=wv.tensor,
                                        offset=wv.offset,
                                        ap=[wv.ap[0], [0, C], wv.ap[1]],
                                    )
                                    nc.vector.tensor_tensor(
                                        prod[:rr, j, :, :Wc], wb, xv, op=OP.mult
                                    )
                                    est.dve(C * Wc, acc=acc2)
                                else:
                                    for c in range(C):
                                        nc.vector.tensor_tensor(
                                            prod[:rr, j, c, :Wc], wv, xv[:, c, :], op=OP.mult
                                        )
                                        est.dve(Wc, acc=acc2)
                            for j, (wv, sh, rr) in enumerate(
                                ((w_d, oy, Re), (w_m, 0, R))
                            ):
                                stop = last_tap and j == 1
                                lhsT = sid(sh, k2v)
                                if cfg.pack_mm:
                                    nc.tensor.matmul(
                                        num[:R, :, :Wc],
                                        lhsT[:rr, :R],
                                        prod[:rr, j, :, :Wc],
                                        start=False,
                                        stop=stop,
                                        skip_group_check=True,
                                    )
                                else:
                                    for c in range(C):
                                        nc.tensor.matmul(
                                            num[:R, c, :Wc],
                                            lhsT[:rr, :R],
                                            prod[:rr, j, c, :Wc],
                                            start=False,
                                            stop=stop,
                                            skip_group_check=True,
                                        )
                                nc.tensor.matmul(
                                    den[:R, :Wc],
                                    lhsT[:rr, :R],
                                    wv,
                                    start=False,
                                    stop=stop,
                                    skip_group_check=True,
                                )
                                est.pe(Wc, n=4)

                        rden = work.tile([128, WE_MAX], f32, tag="rden")
                        if cfg.recip == "fast":
                            nc.vector.reciprocal_approx_fast(rden[:R, :Wc], den[:R, :Wc])
                        else:
                            nc.vector.reciprocal(rden[:R, :Wc], den[:R, :Wc])
                        est.dve(Wc, acc=1)
                        outt = io.tile([128, C, WE_MAX], dt_out, tag="outt")
                        if cfg.fast_final and dt_out != f32:
                            # hoist the f32->bf16 rounds onto ScalarE so the
                            # final mult has all-SBUF 2-byte operands (DVE 2x)
                            rden_b = work.tile([128, WE_MAX], dt, tag="rdenb")
                            nc.scalar.activation(
                                rden_b[:R, :Wc], rden[:R, :Wc], AF.Identity
                            )
                            numb = work.tile([128, C, WE_MAX], dt, tag="numb")
                            nc.scalar.activation(
                                numb[:R, :, :Wc], num[:R, :, :Wc], AF.Identity
                            )
                            est.act(Wc)
                            est.act(C * Wc)
                            rsl = rden_b[:R, :Wc]
                            rb = bass.AP(
                                tensor=rsl.tensor,
                                offset=rsl.offset,
                                ap=[list(rsl.ap[0]), [0, C], list(rsl.ap[1])],
                            )
                            nc.vector.tensor_tensor(
                                outt[:R, :, :Wc], numb[:R, :, :Wc], rb, op=OP.mult
                            )
                            est.dve(C * Wc, acc=acc2)
                        else:
                            rsl = rden[:R, :Wc]
                            rb = bass.AP(
                                tensor=rsl.tensor,
                                offset=rsl.offset,
                                ap=[list(rsl.ap[0]), [0, C], list(rsl.ap[1])],
                            )
                            nc.vector.tensor_tensor(
                                outt[:R, :, :Wc], num[:R, :, :Wc], rb, op=OP.mult
                            )
                            est.dve(C * Wc, acc=1)
                        # reference semantics: out-of-image cols of the
                        # intermediate are edge replicas. Overwrite the bands
                        # in-tile (bias-broadcast copy) before storing.
                        if it < n - 1 and pads[it + 1] > 0:
                            pn = pads[it + 1]
                            if wci == 0:
                                for c in range(C):
                                    nc.scalar.activation(
                                        outt[:R, c, 0:pn], outt[:R, c, 0:pn],
                                        AF.Identity, scale=0.0,
                                        bias=outt[:R, c, pn : pn + 1],
                                    )
                            if wci == ncw - 1:
                                for c in range(C):
                                    nc.scalar.activation(
                                        outt[:R, c, Wc - pn : Wc], outt[:R, c, Wc - pn : Wc],
                                        AF.Identity, scale=0.0,
                                        bias=outt[:R, c, Wc - pn - 1 : Wc - pn],
                                    )
                        nc.sync.dma_start(
                            out=dst[:, r0 : r0 + R, c0 : c0 + Wc].rearrange(
                                "c r w -> r c w"
                            ),
                            in_=outt[:R, :, :Wc],
                        )
                        est.dma(R * C * Wc * (4 if it == n - 1 else esz))

                # reference semantics: out-of-image rows of the intermediate
                # are edge replicas (row-broadcast DMA; innermost dim stays
                # contiguous so DGE accepts it). Cols were fixed in-tile.
                if it < n - 1:
                    pk = pads[it + 1]
                    if pk > 0:
                        def _bcast(src_ap, axis_idx, count):
                            ap = [list(a) for a in src_ap.ap]
                            ap[axis_idx] = [0, count]
                            return bass.AP(tensor=src_ap.tensor, offset=src_ap.offset, ap=ap)

                        # top rows [0, pk) := row pk
                        nc.sync.dma_start(
                            out=dst[:, 0:pk, :],
                            in_=_bcast(dst[:, pk : pk + 1, :], 1, pk),
                        )
                        if cfg.replicate_bottom:
                            nc.sync.dma_start(
                                out=dst[:, Ho - pk : Ho, :],
                                in_=_bcast(dst[:, Ho - pk - 1 : Ho - pk, :], 1, pk),
                            )

    if cfg.report:
        est.show(f"n={n} dt={cfg.dt_elem} sqdiff={cfg.sqdiff} d_reduce={cfg.d_reduce} reps={cfg.reps}")
    nc.compile()
    return nc


# ---------------------------------------------------------------------------
# host side
# ---------------------------------------------------------------------------


def _shard_and_run(inp: np.ndarray, param: np.ndarray, cfg: Cfg):
    import ml_dtypes
    from concourse.bass_utils import run_bass_kernel_spmd

    B, C, Hfull, Wfull = inp.shape
    n = cfg.numIter
    pad0 = _pads(n)[0]
    nh = 8 // B
    assert nh in (1, 2), "sharding assumes each shard has at most one global row-edge"
    Hsh = Hfull // nh
    assert Hsh == cfg.Hout and Wfull == cfg.Wout

    np_dt = np.float32 if cfg.dt_elem == "float32" else ml_dtypes.bfloat16
    xp = np.pad(inp, ((0, 0), (0, 0), (pad0, pad0), (pad0, pad0)), mode="edge").astype(np_dt)
    gp = np.pad(param, ((0, 0), (0, 0), (pad0, pad0), (pad0, pad0)), mode="edge").astype(np_dt)

    in_maps = []
    for b in range(B):
        for h in range(nh):
            r = h * Hsh
            xs = xp[b, :, r : r + Hsh + 2 * pad0, :]
            gs = gp[b, :, r : r + Hsh + 2 * pad0, :]
            if h == nh - 1 and nh > 1:
                # flip so the global row-edge is at the top of the shard
                xs, gs = xs[:, ::-1, :], gs[:, ::-1, :]
            in_maps.append(
                {"x": np.ascontiguousarray(xs), "g": np.ascontiguousarray(gs)}
            )

    nc = build(cfg)
    res = run_bass_kernel_spmd(nc, in_maps, core_ids=list(range(8)))

    out = np.empty((B, C, Hfull, Wfull), dtype=np.float32)
    for b in range(B):
        for h in range(nh):
            o = res.results[b * nh + h]["out"]
            if h == nh - 1 and nh > 1:
                o = o[:, ::-1, :]
            out[b, :, h * Hsh : (h + 1) * Hsh, :] = o
    return out


def kernel(input: np.ndarray, param: np.ndarray, numIter) -> np.ndarray:
    n = int(numIter)
    inp = np.asarray(input, dtype=np.float32)
    g = np.asarray(param, dtype=np.float32)
    if n <= 0:
        return inp.copy()
    nh = 8 // inp.shape[0]
    cfg = Cfg(
        Hout=inp.shape[2] // nh,
        Wout=inp.shape[3],
        C=inp.shape[1],
        P=g.shape[1],
        numIter=n,
        replicate_bottom=(nh == 1),
    )
    return _shard_and_run(inp, g, cfg)

